# revision 1
# baseline (speedup 1.0000x reference)
"""Trainium2 Bass kernel for nn_MultiHeadAttention (dense transformer block).

Reference computation (per batch b of B=2, N=2048 tokens, C=1024, H=16 heads,
D=64 head dim):
    qkv  = x @ W_qkv.T + b_qkv
    q,k,v split into heads; attn = softmax(q @ k.T / sqrt(D)); o = attn @ v
    out  = o @ W_proj.T + b_proj

Sharding over 8 NeuronCores: batch x head-groups.  Core c handles batch
b = c//4 and the 4 heads [4*(c%4), 4*(c%4)+4).  Attention is computed fully
per (batch, head) on one core.  The output projection needs all heads, so
cores AllGather their head-group outputs O^T (f32r) within their 4-core
batch group, then each core computes the full projection for a distinct
512-token slice of its batch.  Host concatenates the 8 slices.

All matmuls run as float32r (full-speed fp32 path on the PE).
"""

import sys

sys.path.insert(0, "/opt/trn_rl_repo")

import numpy as np
import concourse.bass as bass
import concourse.tile as tile
from concourse import mybir, bacc
from concourse.bass_utils import run_bass_kernel_spmd

f32 = mybir.dt.float32
f32r = mybir.dt.float32r

# problem constants (hardcoded per contract)
B = 2
N = 2048
C = 1024
H = 16
D = C // H  # 64
SCALE = D ** -0.5

NCORES = 8
GROUPS = [[0, 1, 2, 3], [4, 5, 6, 7]]
HPC = H // 4  # heads per core = 4
ODC = HPC * D  # per-core o-dim slice = 256
TOKS = N // 4  # output token slice per core = 512


def build_kernel(n=N, c_dim=C, hpc=HPC, ag=True, phases=(1, 2, 4), reps=1):
    """Builds the per-core Bass program. n = sequence length, c_dim = model
    dim, hpc = heads per core (4).  Shapes below follow the real problem when
    defaults are used; smaller n can be used for simulator checks."""
    d = D
    odc = hpc * d                      # 256: per-core o dims
    n_ct = c_dim // 128                # contraction chunks for C
    n_ic = n // 512                    # i (query) chunks of 512
    n_jt = n // 128                    # j (key) tiles of 128
    n_jp = n_jt // 2                   # j tile pairs
    toks = n // 4                      # per-core output token slice
    s_chunks = odc // 128              # 128-wide stationary chunks for q/k/v (2)

    nc = bacc.Bacc("TRN2", target_bir_lowering=False, debug=False,
                   num_devices=NCORES)

    # ---- DRAM I/O ----
    xt = nc.dram_tensor("xt", [c_dim, n], f32r, kind="ExternalInput").ap()
    wq_t = nc.dram_tensor("wq_t", [c_dim, odc], f32r, kind="ExternalInput").ap()
    wk_t = nc.dram_tensor("wk_t", [c_dim, odc], f32r, kind="ExternalInput").ap()
    wv_t = nc.dram_tensor("wv_t", [c_dim, odc], f32r, kind="ExternalInput").ap()
    bqkv = nc.dram_tensor("bqkv", [128, 3 * s_chunks], f32, kind="ExternalInput").ap()
    wp_t = nc.dram_tensor("wp_t", [4 * odc, c_dim], f32r, kind="ExternalInput").ap()
    bp = nc.dram_tensor("bp", [c_dim], f32, kind="ExternalInput").ap()
    y = nc.dram_tensor("y", [toks, c_dim], f32, kind="ExternalOutput").ap()

    with tile.TileContext(nc, pool_alloc_mode="queue") as tc:
        with (
            tc.tile_pool(name="consts", bufs=1) as consts,
            tc.tile_pool(name="qkvsb", bufs=1) as qkvsb,
            tc.tile_pool(name="dram", bufs=1, space="DRAM") as dram,
        ):
            # ---- constants ----
            bqkv_sb = consts.tile([128, 3 * s_chunks], f32)
            nc.sync.dma_start(out=bqkv_sb, in_=bqkv)
            ones32 = consts.tile([128, 1], f32)
            nc.vector.memset(ones32, 1.0)
            ones_r = consts.tile([1, 64], f32r)
            nc.vector.tensor_copy(out=ones_r, in_=ones32[0:1, 0:1].to_broadcast((1, 64)))
            # identity blocks at both partition halves (transpose lhsT base
            # partition must match the identity's)
            ident = consts.tile([128, 64], f32)
            nc.gpsimd.memset(ident, 0.0)
            for half in range(2):
                nc.gpsimd.affine_select(
                    out=ident[half * 64:(half + 1) * 64, :],
                    in_=ident[half * 64:(half + 1) * 64, :],
                    compare_op=mybir.AluOpType.not_equal,
                    fill=1.0, base=0, pattern=[[-1, 64]], channel_multiplier=1,
                )

            # persistent SBUF activations
            qt_sb = qkvsb.tile([128, s_chunks, n], f32r)   # q^T  (head h -> (s=h//2, half=h%2))
            kt_sb = qkvsb.tile([128, s_chunks, n], f32r)   # k^T
            vp_sb = qkvsb.tile([128, n_jt, hpc, 65], f32r)  # v natural + ones col
            ot_sb = qkvsb.tile([128, s_chunks, n], f32r)   # o^T (unnorm->normed)

            # ones column of V'
            nc.vector.tensor_copy(
                out=vp_sb[:, :, :, 64:65],
                in_=ones32[:, 0:1].to_broadcast((128, n_jt, hpc, 1)),
            )

            # ---------- Phase 1: QKV^T projections ----------
            if 1 not in phases:
                for _t in (qt_sb, kt_sb, ot_sb):
                    nc.vector.tensor_copy(out=_t[:, :, 0:1],
                                          in_=ones32[:, 0:1].to_broadcast((128, _t.shape[1], 1)))
                nc.vector.tensor_copy(out=vp_sb[:, 0, :, 0:1],
                                      in_=ones32[:, 0:1].to_broadcast((128, vp_sb.shape[2], 1)))
            if 1 in phases:
              with (
                tc.tile_pool(name="p1w", bufs=1) as p1w,
                tc.tile_pool(name="p1x", bufs=1) as p1x,
                tc.tile_pool(name="p1ps", bufs=3, space="PSUM") as p1ps,
                tc.tile_pool(name="p1tr", bufs=2, space="PSUM") as p1tr,
                tc.tile_pool(name="p1vt", bufs=1) as p1vt,
            ):
                xt_sb = p1x.tile([128, n_ct, n], f32r)
                xt_v = xt.rearrange("(ct p) n -> p ct n", p=128)
                w_sb = {}
                w_vs = {}
                for name, t in (("q", wq_t), ("k", wk_t), ("v", wv_t)):
                    w_sb[name] = p1w.tile([128, n_ct, odc], f32r, name=f"w_{name}")
                    w_vs[name] = t.rearrange("(ct p) m -> p ct m", p=128)
                for ct in range(n_ct):
                    for name in ("q", "k", "v"):
                        nc.sync.dma_start(out=w_sb[name][:, ct, :],
                                          in_=w_vs[name][:, ct, :])
                    nc.sync.dma_start(out=xt_sb[:, ct, :], in_=xt_v[:, ct, :])
                vt_sb = p1vt.tile([128, s_chunks, n], f32)  # v^T staging

                for ti, tname in enumerate(("q", "k", "v")):
                    dst = (qt_sb, kt_sb, vt_sb)[ti]
                    for s in range(s_chunks):
                        bias_col = ti * s_chunks + s
                        for half in range(n // 1024):
                            ps = p1ps.tile([128, 1024], f32, tag="p1ps")
                            for ct in range(n_ct):
                                for n2 in range(2):
                                    nt = half * 2 + n2
                                    nc.tensor.matmul(
                                        ps[:, n2 * 512:(n2 + 1) * 512],
                                        lhsT=w_sb[tname][:, ct, s * 128:(s + 1) * 128],
                                        rhs=xt_sb[:, ct, nt * 512:(nt + 1) * 512],
                                        start=(ct == 0), stop=(ct == n_ct - 1),
                                    )
                            nc.vector.tensor_scalar_add(
                                out=dst[:, s, half * 1024:(half + 1) * 1024],
                                in0=ps,
                                scalar1=bqkv_sb[:, bias_col:bias_col + 1],
                            )

                # transpose V^T -> V natural blocks into vp_sb
                for s in range(s_chunks):  # keep indent
                    for hh in range(2):
                        h_loc = s * 2 + hh
                        for jt in range(n_jt):
                            ptr = p1tr.tile([128, 64], f32, tag="p1tr")
                            nc.tensor.transpose(
                                ptr,
                                in_=vt_sb[hh * 64:(hh + 1) * 64, s,
                                          jt * 128:(jt + 1) * 128],
                                identity=ident[hh * 64:(hh + 1) * 64, :],
                            )
                            nc.vector.tensor_copy(
                                out=vp_sb[:, jt, h_loc, 0:64], in_=ptr
                            )

            # ---------- Phase 2: attention per head ----------
            for _rep in range(reps):
              ag_outs = []
              if 2 in phases:
                with (
                  tc.tile_pool(name="p2s", bufs=2, space="PSUM") as p2s,
                  tc.tile_pool(name="p2u", bufs=2, space="PSUM") as p2u,
                  tc.tile_pool(name="p2b", bufs=2, space="PSUM") as p2b,
                  tc.tile_pool(name="p2e", bufs=6) as p2e,
                  tc.tile_pool(name="p2r", bufs=2) as p2r,
              ):
                  for s in range(s_chunks):
                      for ic in range(n_ic):
                          ps_u = [p2u.tile([65, 512], f32, tag="p2u", name=f"ps_u{_h}") for _h in range(2)]
                          for jp in range(n_jp):
                              e_t = []
                              for hh in range(2):
                                  ps_s = p2s.tile([128, 1024], f32, tag="p2s")
                                  for j2 in range(2):
                                      jt = jp * 2 + j2
                                      nc.tensor.matmul(
                                          ps_s[:, j2 * 512:(j2 + 1) * 512],
                                          lhsT=kt_sb[hh * 64:(hh + 1) * 64, s,
                                                     jt * 128:(jt + 1) * 128],
                                          rhs=qt_sb[hh * 64:(hh + 1) * 64, s,
                                                    ic * 512:(ic + 1) * 512],
                                          start=True, stop=True,
                                      )
                                  e = p2e.tile([128, 1024], f32r, tag="p2e")
                                  nc.scalar.activation(
                                      out=e, in_=ps_s,
                                      func=mybir.ActivationFunctionType.Exp,
                                  )
                                  e_t.append(e)
                              for hh in range(2):
                                  for j2 in range(2):
                                      jt = jp * 2 + j2
                                      nc.tensor.matmul(
                                          ps_u[hh],
                                          lhsT=vp_sb[:, jt, s * 2 + hh, :],
                                          rhs=e_t[hh][:, j2 * 512:(j2 + 1) * 512],
                                          start=(jp == 0 and j2 == 0),
                                          stop=(jp == n_jp - 1 and j2 == 1),
                                      )
                          for hh in range(2):  # normalize
                              r32 = p2r.tile([1, 512], f32, tag="r32")
                              nc.vector.reciprocal(out=r32, in_=ps_u[hh][64:65, :])
                              rr = p2r.tile([1, 512], f32r, tag="rr")
                              nc.vector.tensor_copy(out=rr, in_=r32)
                              ps_b = p2b.tile([64, 512], f32, tag="p2b")
                              nc.tensor.matmul(ps_b, lhsT=ones_r, rhs=rr,
                                               start=True, stop=True)
                              rb_sb = p2r.tile([64, 512], f32, tag="rb_sb")
                              nc.vector.tensor_copy(out=rb_sb, in_=ps_b)
                              nc.vector.tensor_mul(
                                  out=ot_sb[hh * 64:(hh + 1) * 64, s,
                                            ic * 512:(ic + 1) * 512],
                                  in0=ps_u[hh][0:64, :],
                                  in1=rb_sb,
                              )
                      if ag:
                          ag_in_s = dram.tile([128, n], f32r,
                                              name=f"ag_in{s}_{_rep}")
                          nc.sync.dma_start(out=ag_in_s, in_=ot_sb[:, s, :])
                          ag_out_s = dram.tile([512, n], f32r,
                                               name=f"ag_out{s}_{_rep}")
                          nc.gpsimd.collective_compute(
                              "AllGather",
                              mybir.AluOpType.bypass,
                              ins=[ag_in_s[:].opt()],
                              outs=[ag_out_s[:].opt()],
                              replica_groups=GROUPS,
                          )
                          ag_outs.append(ag_out_s)

              # ---------- Phase 3: AllGather O^T ----------
              if 2 not in phases and 4 not in phases:
                  nc.sync.dma_start(out=y.bitcast(f32r), in_=qt_sb[:, :, 0:(toks * c_dim) // (128 * s_chunks)])
                  continue
              if not ag:
                  ag_in = dram.tile([odc, n], f32r)
                  nc.sync.dma_start(
                      out=ag_in.rearrange("(s p) n -> p s n", p=128), in_=ot_sb
                  )
                  ag_out = ag_in

              # ---------- Phase 4: projection on own token slice ----------
              if 4 in phases:
                with (
                  tc.tile_pool(name="p4o", bufs=1) as p4o,
                  tc.tile_pool(name="p4w", bufs=1) as p4w,
                  tc.tile_pool(name="p4ps", bufs=4, space="PSUM") as p4ps,
                  tc.tile_pool(name="p4y", bufs=3) as p4y,
              ):
                  n_od = (4 * odc) // 128 if ag else odc // 128
                  # own token slice: (partition_id % 4) * toks .. +toks
                  ogt = p4o.tile([128, n_od, toks], f32r)
                  pid = nc.partition_id()
                  tok0 = (pid % 4) * toks
                  if ag:
                      ogt_v = ogt.rearrange("p (g s) t -> p g s t", s=s_chunks)
                      for s in range(s_chunks):
                          nc.sync.dma_start(
                              out=ogt_v[:, :, s, :],
                              in_=ag_outs[s].rearrange("(g p) n -> p g n", p=128)[
                                  :, :, bass.ds(tok0, toks)
                              ],
                          )
                  else:
                      nc.sync.dma_start(
                          out=ogt,
                          in_=ag_out.rearrange("(od p) n -> p od n", p=128)[
                              :, :, bass.ds(tok0, toks)
                          ],
                      )
                  wp_sb = p4w.tile([128, n_od, c_dim], f32r)
                  nc.sync.dma_start(
                      out=wp_sb, in_=wp_t.rearrange("(od p) c -> p od c", p=128)[
                          :, 0:n_od, :
                      ]
                  )
                  bp_bc = p4w.tile([128, c_dim], f32)
                  nc.sync.dma_start(
                      out=bp_bc,
                      in_=bass.AP(tensor=bp.tensor, offset=bp.offset,
                                  ap=[[0, 128]] + bp.ap),
                  )
                  for tt in range(toks // 128):
                      ps_y = [p4ps.tile([128, 512], f32, tag="p4ps", name=f"ps_y{_n}") for _n in range(c_dim // 512)]
                      for nc2 in range(c_dim // 512):
                          for od in range(n_od):
                              nc.tensor.matmul(
                                  ps_y[nc2],
                                  lhsT=ogt[:, od, tt * 128:(tt + 1) * 128],
                                  rhs=wp_sb[:, od, nc2 * 512:(nc2 + 1) * 512],
                                  start=(od == 0), stop=(od == n_od - 1),
                              )
                      y_sb = p4y.tile([128, c_dim], f32, tag="y_sb")
                      for nc2 in range(c_dim // 512):
                          nc.vector.tensor_add(
                              out=y_sb[:, nc2 * 512:(nc2 + 1) * 512],
                              in0=ps_y[nc2],
                              in1=bp_bc[:, nc2 * 512:(nc2 + 1) * 512],
                          )
                      nc.sync.dma_start(out=y[tt * 128:(tt + 1) * 128, :], in_=y_sb)

    nc.compile()
    return nc


_CACHE = {}


def _get_nc():
    if "nc" not in _CACHE:
        _CACHE["nc"] = build_kernel()
    return _CACHE["nc"]


def make_in_maps(x, W_qkv, b_qkv, W_proj, b_proj):
    x = np.asarray(x, dtype=np.float32)
    W_qkv = np.asarray(W_qkv, dtype=np.float32)
    b_qkv = np.asarray(b_qkv, dtype=np.float32)
    W_proj = np.asarray(W_proj, dtype=np.float32)
    b_proj = np.asarray(b_proj, dtype=np.float32)

    Wq = W_qkv[0:C] * SCALE
    Wk = W_qkv[C:2 * C]
    Wv = W_qkv[2 * C:3 * C]
    bq = b_qkv[0:C] * SCALE
    bk = b_qkv[C:2 * C]
    bv = b_qkv[2 * C:3 * C]
    wp_t_full = np.ascontiguousarray(W_proj.T)  # [C(od), C]

    # host-side layout prep, deduplicated: x^T is shared by the 4 cores of
    # a batch; weight slices are shared by the 2 cores of a head-group
    xt_by_b = [np.ascontiguousarray(x[b].T) for b in range(B)]  # [C, N]
    per_g = []
    for g in range(4):
        rows = slice(g * ODC, (g + 1) * ODC)
        bcols = np.stack(
            [bq[rows][0:128], bq[rows][128:256],
             bk[rows][0:128], bk[rows][128:256],
             bv[rows][0:128], bv[rows][128:256]], axis=1
        )  # [128, 6]
        per_g.append({
            "wq_t": np.ascontiguousarray(Wq[rows].T),
            "wk_t": np.ascontiguousarray(Wk[rows].T),
            "wv_t": np.ascontiguousarray(Wv[rows].T),
            "bqkv": np.ascontiguousarray(bcols),
        })
    in_maps = []
    for core in range(NCORES):
        b = core // 4
        g = core % 4
        in_maps.append({
            "xt": xt_by_b[b],
            **per_g[g],
            "wp_t": wp_t_full,
            "bp": b_proj,
        })
    return in_maps


def kernel(x, W_qkv, b_qkv, W_proj, b_proj):
    x = np.asarray(x, dtype=np.float32)
    nc = _get_nc()
    in_maps = make_in_maps(x, W_qkv, b_qkv, W_proj, b_proj)
    res = run_bass_kernel_spmd(nc, in_maps, list(range(NCORES)))

    out = np.empty((B, N, C), dtype=np.float32)
    for core in range(NCORES):
        b = core // 4
        g = core % 4
        out[b, g * TOKS:(g + 1) * TOKS, :] = res.results[core]["y"]
    return out

